# revision 26
# baseline (speedup 1.0000x reference)
"""Trainium2 Bass kernel for the ATFA dense-transformer problem.

Shapes (hardcoded): x [2, 249, 64, 256]; two attention blocks (freq: attend
over T per (b,f) head; time: attend over F per (b,t) head), each preceded by
3x3 'SAME' q/k/v convs; gated concat + final 3x3 conv to 64 channels.

Sharding across 8 cores, one uniform NEFF, no collectives:
- freq path F-sharded: core i computes heads f in [8i-1, 8i+9) (halo heads
  duplicated), full T, then the alpha-gated part of the final conv for its own
  8 F-columns.
- time path T-sharded: core i computes time-attn for rows [32i-1, 32i+33),
  then the beta-gated + x parts of the final conv for rows [32i, 32i+32).
Host zero-pads every slice (uniform shapes; SAME-conv padding falls out) and
sums the two partial conv outputs.

All matmul data is float32r (FP22 multiply, fp32 accumulate): full PE rate at
output free size >= 256.
"""

import os
import numpy as np

B, T, F, C = 2, 249, 64, 256
OUT_CH = 64
TP = 258          # padded time axis for freq path: tt = t + 1, t in [-1, 257)
TQ = 256          # padded T for q/k free dims
FH = 10           # xf freq columns: global f in [8i-1, 8i+9)
NH = 8            # freq heads per core: f in [8i, 8i+8) (owned only)
FO = 10           # partial final-conv cols: global f in [8i-1, 8i+9)
TH = 38           # xt time rows: tx = t - 32i + 2, global t in [32i-2, 32i+36)
FP = 66           # padded F axis for time path: fp = f + 1
NTH = 34          # time heads per core: tx in [1, 35) -> t in [32i-1, 32i+33)
R_OWN = 32        # owned time rows per core

_CACHE = {}


def _build_program():
    import concourse.bass as bass
    import concourse.mybir as mybir
    import concourse.tile as tile
    from concourse import bacc

    F32 = mybir.dt.float32
    F32R = mybir.dt.float32r
    EXP = mybir.ActivationFunctionType.Exp
    AX = mybir.AxisListType.X

    nc = bacc.Bacc("TRN2", target_bir_lowering=False, debug=False)

    # ---- DRAM I/O (all float32r; host numpy side is float32) ----
    xf_d = nc.dram_tensor("xf", [C, B, FH, TP], F32R, kind="ExternalInput")
    xt_d = nc.dram_tensor("xt", [C, B, TH, FP], F32R, kind="ExternalInput")
    al_d = nc.dram_tensor("al", [C, B, NH, TQ], F32R, kind="ExternalInput")
    be_d = nc.dram_tensor("be", [C, B, NTH, F], F32R, kind="ExternalInput")
    # weights, channel-major: [cin 256, tap 9, cout]
    wdecl = {}
    for name, co in [("wqf", C), ("wkf", C), ("wvf", C),
                     ("wqt", C), ("wkt", C), ("wvt", C),
                     ("wff", OUT_CH), ("wft", OUT_CH), ("wfx", OUT_CH)]:
        wdecl[name] = nc.dram_tensor(name, [C, 9, co], F32R, kind="ExternalInput")
    # per-cout-chunk biases [2, 128, 1] for q/k/v convs
    bdecl = {}
    for name in ["bqf", "bkf", "bvf", "bqt", "bkt", "bvt"]:
        bdecl[name] = nc.dram_tensor(name, [2, 128, 1], F32, kind="ExternalInput")
    id_d = nc.dram_tensor("ident", [128, 128], F32R, kind="ExternalInput")
    zz_d = nc.dram_tensor("zz", [128, 256], F32R, kind="ExternalInput")
    of_d = nc.dram_tensor("of", [FO, OUT_CH, B, TQ], F32R, kind="ExternalOutput")
    ot_d = nc.dram_tensor("ot", [8, OUT_CH, B, 4, F], F32R, kind="ExternalOutput")

    with tile.TileContext(nc) as tc:
        with (
            tc.tile_pool(name="glob", bufs=1) as glob,
            tc.tile_pool(name="ps_out", bufs=2, space="PSUM") as ps_out,
        ):
            ident = glob.tile([128, 128], F32R)
            # final-conv weights live through both phases
            wff = glob.tile([128, 2, 9, OUT_CH], F32R)
            wft = glob.tile([128, 2, 9, OUT_CH], F32R)
            wfx = glob.tile([128, 2, 9, OUT_CH], F32R)
            wqt = glob.tile([128, 2, 9, C], F32R)

            # ============== PHASE A: freq path ==============
            with (
                tc.tile_pool(name="pa", bufs=1) as pa,
                tc.tile_pool(name="pa2", bufs=2) as pa2,
                tc.tile_pool(name="pa3", bufs=3) as pa3,
                tc.tile_pool(name="ps_conv", bufs=2, space="PSUM") as ps_conv,
                tc.tile_pool(name="ps_attn", bufs=4, space="PSUM") as ps_attn,
            ):
                xf = pa.tile([128, 2, B, FH, TP], F32R, tag="xf")
                xf_src = xf_d.rearrange("(cc p) b f t -> p cc b f t", p=128)
                wqf = pa.tile([128, 2, 9, C], F32R, tag="wqf")
                wkf = pa.tile([128, 2, 9, C], F32R, tag="wkf")
                wvf = pa.tile([128, 2, 9, C], F32R, tag="wvf")
                bqf = pa.tile([128, 2, 1], F32, tag="bqf")
                bkf = pa.tile([128, 2, 1], F32, tag="bkf")
                bvf = pa.tile([128, 2, 1], F32, tag="bvf")
                # ordering: first conv (q, head 0) needs wqf + bqf + xf cols
                # 0-2; weights go on the gpsimd queue, xf on the sync queue.
                for t_, n_ in [(bqf, "bqf"), (bkf, "bkf"), (bvf, "bvf")]:
                    nc.sync.dma_start(
                        out=t_, in_=bdecl[n_].rearrange("c p o -> p c o"))
                wsrcs = {n_: wdecl[n_].rearrange("(cc p) t o -> p cc t o", p=128)
                         for n_ in ("wqf", "wkf", "wvf")}
                for cc2 in (0, 1):
                    nc.sync.dma_start(out=wqf[:, cc2], in_=wsrcs["wqf"][:, cc2])
                    for fx in range(3):
                        nc.sync.dma_start(out=xf[:, cc2, :, fx, :],
                                          in_=xf_src[:, cc2, :, fx, :])
                nc.sync.dma_start(out=ident, in_=id_d[:])
                for cc2 in (0, 1):
                    nc.sync.dma_start(out=wkf[:, cc2], in_=wsrcs["wkf"][:, cc2])
                    nc.sync.dma_start(out=wvf[:, cc2], in_=wsrcs["wvf"][:, cc2])
                for fx in range(3, FH):
                    for cc2 in (0, 1):
                        nc.sync.dma_start(out=xf[:, cc2, :, fx, :],
                                          in_=xf_src[:, cc2, :, fx, :])

                for t_, d_ in [(wff, wdecl["wff"]), (wft, wdecl["wft"]),
                               (wfx, wdecl["wfx"])]:
                    nc.sync.dma_start(
                        out=t_, in_=d_.rearrange("(cc p) t o -> p cc t o", p=128))
                wqt_src = wdecl["wqt"].rearrange("(cc p) t o -> p cc t o", p=128)
                for cc2 in (0, 1):
                    nc.sync.dma_start(out=wqt[:, cc2], in_=wqt_src[:, cc2])

                # gated alpha*out_freq, layout [c, b, fl, tt]; zero tt=0 col
                gf = [pa.tile([128, B, NH, TP], F32R, tag=f"gf{cc}", name=f"gf{cc}") for cc in (0, 1)]
                for cc in (0, 1):
                    for col in (0, TP - 1):
                        nc.sync.dma_start(
                            out=gf[cc][:, :, :, col:col + 1],
                            in_=zz_d[:, 0:B * NH].rearrange(
                                "p (b f o) -> p b f o", b=B, f=NH))

                for fl in range(NH):
                    # --- q/k/v convs for head column fl, both batches ---
                    sb_qkv = []
                    for wt, bs, nm in [(wqf, bqf, "q"), (wkf, bkf, "k"),
                                       (wvf, bvf, "v")]:
                        sb = pa2.tile([128, 2, B, TQ], F32R, tag=f"sb_{nm}")
                        for oc in (0, 1):
                            ps = ps_conv.tile([128, B, TQ], F32, tag="conv")
                            n = 0
                            for ccin in (0, 1):
                                for i in range(3):
                                    for j in range(3):
                                        nc.tensor.matmul(
                                            ps[:],
                                            wt[:, ccin, 3 * i + j,
                                               128 * oc:128 * (oc + 1)],
                                            xf[:, ccin, :, fl + i, j:j + TQ],
                                            start=(n == 0), stop=(n == 17))
                                        n += 1
                            nc.vector.tensor_scalar_add(
                                out=sb[:, oc], in0=ps[:], scalar1=bs[:, oc])
                        sb_qkv.append(sb)
                    q_sb, k_sb, v_sb = sb_qkv

                    for b in range(B):
                        # --- scores + softmax (k on free dim) ---
                        dist = pa2.tile([128, 2, TQ], F32R, tag="dist")
                        rstat = pa3.tile([128, 2, 2], F32, tag="rstat")
                        for qc in (0, 1):
                            sps = ps_attn.tile([128, TQ], F32, tag="attn")
                            for oc in (0, 1):
                                nc.tensor.matmul(
                                    sps[:],
                                    q_sb[:, oc, b, 128 * qc:128 * (qc + 1)],
                                    k_sb[:, oc, b, :],
                                    start=(oc == 0), stop=(oc == 1))
                            nc.vector.reduce_max(
                                out=rstat[:, qc, 0:1], in_=sps[:, 0:T],
                                axis=AX, negate=True)
                            nc.scalar.activation(
                                out=dist[:, qc], in_=sps[:], func=EXP,
                                bias=rstat[:, qc, 0:1], scale=1.0,
                                accum_out=rstat[:, qc, 1:2])
                            nc.vector.reciprocal(
                                out=rstat[:, qc, 1:2], in_=rstat[:, qc, 1:2])
                            nc.vector.tensor_scalar_mul(
                                out=dist[:, qc], in0=dist[:, qc],
                                scalar1=rstat[:, qc, 1:2])
                        # --- transpose dist -> [k, q] ---
                        dT = pa2.tile([128, 2, TQ], F32R, tag="dT")
                        for kc in (0, 1):
                            tps = ps_attn.tile([128, TQ], F32R, tag="attn")
                            for qc in (0, 1):
                                nc.tensor.transpose(
                                    tps[:, 128 * qc:128 * (qc + 1)],
                                    dist[:, qc, 128 * kc:128 * (kc + 1)], ident)
                            nc.scalar.copy(out=dT[:, kc], in_=tps[:])
                        # --- transpose v^T[c, t] -> V[t, c] ---
                        vT = pa2.tile([128, 2, TQ], F32R, tag="vT")
                        for tc2 in (0, 1):
                            vps = ps_attn.tile([128, TQ], F32R, tag="attn")
                            for oc in (0, 1):
                                nc.tensor.transpose(
                                    vps[:, 128 * oc:128 * (oc + 1)],
                                    v_sb[:, oc, b, 128 * tc2:128 * (tc2 + 1)],
                                    ident)
                            nc.scalar.copy(out=vT[:, tc2], in_=vps[:])
                        # --- out^T[c, q] = sum_k V[k, c] * dT[k, q]; gate ---
                        alt = pa3.tile([128, 2, TQ], F32R, tag="alt")
                        nc.sync.dma_start(
                            out=alt,
                            in_=al_d.rearrange("(cc p) b f t -> p cc b f t",
                                               p=128)[:, :, b, fl, :])
                        for oc in (0, 1):
                            aps = ps_attn.tile([128, TQ], F32, tag="attn")
                            for kc in (0, 1):
                                nc.tensor.matmul(
                                    aps[:],
                                    vT[:, kc, 128 * oc:128 * (oc + 1)],
                                    dT[:, kc], start=(kc == 0), stop=(kc == 1))
                            nc.vector.tensor_mul(
                                out=gf[oc][:, b, fl, 1:1 + TQ],
                                in0=aps[:], in1=alt[:, oc])

                    # -- freq partial final conv (scatter-add): output col
                    # fo in [-1, 9) uses only locally-owned heads fo+j-1; col
                    # fo is complete after head min(fo+1, 7) --
                    for fo in ([fl - 1] if fl < 7 else [6, 7, 8]):
                        js = [j for j in range(3) if 0 <= fo + j - 1 < NH]
                        nmm = 2 * 3 * len(js)
                        ops = ps_out.tile([OUT_CH, B, TQ], F32, tag="fin")
                        n = 0
                        for ccin in (0, 1):
                            for i in range(3):
                                for j in js:
                                    nc.tensor.matmul(
                                        ops[:],
                                        wff[:, ccin, 3 * i + j, :],
                                        gf[ccin][:, :, fo + j - 1, i:i + TQ],
                                        start=(n == 0), stop=(n == nmm - 1))
                                    n += 1
                        osb = pa3.tile([OUT_CH, B, TQ], F32R, tag="osb")
                        nc.vector.tensor_copy(out=osb, in_=ops[:])
                        nc.sync.dma_start(out=of_d[fo + 1], in_=osb)

            # ============== PHASE C: time path ==============
            with (
                tc.tile_pool(name="pc", bufs=1) as pc,
                tc.tile_pool(name="pc2", bufs=2) as pc2,
                tc.tile_pool(name="pc3", bufs=3) as pc3,
                tc.tile_pool(name="ps_convc", bufs=2, space="PSUM") as ps_convc,
                tc.tile_pool(name="ps_attnc", bufs=4, space="PSUM") as ps_attnc,
            ):
                xt = pc.tile([128, 2, B, TH, FP], F32R, tag="xt")
                xt_src = xt_d.rearrange("(cc p) b t f -> p cc b t f", p=128)
                wkt = pc.tile([128, 2, 9, C], F32R, tag="wkt")
                wvt = pc.tile([128, 2, 9, C], F32R, tag="wvt")
                bqt = pc.tile([128, 2, 1], F32, tag="bqt")
                bkt = pc.tile([128, 2, 1], F32, tag="bkt")
                bvt = pc.tile([128, 2, 1], F32, tag="bvt")
                for t_, n_ in [(bqt, "bqt"), (bkt, "bkt"), (bvt, "bvt")]:
                    nc.sync.dma_start(
                        out=t_, in_=bdecl[n_].rearrange("c p o -> p c o"))
                wsrcs_t = {n_: wdecl[n_].rearrange("(cc p) t o -> p cc t o", p=128)
                           for n_ in ("wkt", "wvt")}

                def load_xt_rows(r_, re_):
                    for cc2 in (0, 1):
                        for b2 in range(B):
                            nc.sync.dma_start(
                                out=xt[:, cc2, b2, r_:re_, :],
                                in_=xt_src[:, cc2, b2, r_:re_, :])

                load_xt_rows(0, 8)
                for cc2 in (0, 1):
                    nc.sync.dma_start(out=wkt[:, cc2], in_=wsrcs_t["wkt"][:, cc2])
                    nc.sync.dma_start(out=wvt[:, cc2], in_=wsrcs_t["wvt"][:, cc2])
                for r_ in range(8, TH, 8):
                    load_xt_rows(r_, min(TH, r_ + 8))

                gt = [pc.tile([128, B, TH, FP], F32R, tag=f"gt{cc}", name=f"gt{cc}") for cc in (0, 1)]
                for cc in (0, 1):
                    for col in (0, FP - 1):
                        nc.sync.dma_start(
                            out=gt[cc][:, :, :, col:col + 1],
                            in_=zz_d[:, 0:B * TH].rearrange(
                                "p (b t o) -> p b t o", b=B, t=TH))

                # conv groups: rows tx in [1, 35): 8 groups of 4 + 1 of 2
                for g in range(9):
                    r0 = 1 + 4 * g
                    nr = 4 if g < 8 else 2
                    sb_qkv = []
                    for wt, bs, nm in [(wqt, bqt, "q"), (wkt, bkt, "k"),
                                       (wvt, bvt, "v")]:
                        sb = pc2.tile([128, 2, B, nr, F], F32R,
                                      tag=f"sbt_{nm}")
                        for oc in (0, 1):
                            ps = ps_convc.tile([128, B, nr, F], F32,
                                               tag="convc")
                            n = 0
                            for ccin in (0, 1):
                                for i in range(3):
                                    for j in range(3):
                                        nc.tensor.matmul(
                                            ps[:],
                                            wt[:, ccin, 3 * i + j,
                                               128 * oc:128 * (oc + 1)],
                                            xt[:, ccin, :,
                                               r0 + i - 1:r0 + i - 1 + nr,
                                               j:j + F],
                                            start=(n == 0), stop=(n == 17))
                                        n += 1
                            nc.vector.tensor_scalar_add(
                                out=sb[:, oc], in0=ps[:], scalar1=bs[:, oc])
                        sb_qkv.append(sb)
                    q_sb, k_sb, v_sb = sb_qkv

                    for b in range(B):
                        for ti in range(nr):
                            tx = r0 + ti
                            if tx < 1 or tx >= 1 + NTH:
                                continue
                            # vT: [f, c] via 2 PE transposes
                            vT = pc3.tile([F, 2, 128], F32R, tag="vT")
                            vps = ps_attnc.tile([F, 2, 128], F32R, tag="attnc")
                            for oc in (0, 1):
                                nc.tensor.transpose(
                                    vps[:, oc], v_sb[:, oc, b, ti, :], ident)
                            nc.scalar.copy(out=vT, in_=vps[:])
                            # scores [f_q, f_k]
                            sps = ps_attnc.tile([F, F], F32, tag="attnc")
                            for oc in (0, 1):
                                nc.tensor.matmul(
                                    sps[:], q_sb[:, oc, b, ti, :],
                                    k_sb[:, oc, b, ti, :],
                                    start=(oc == 0), stop=(oc == 1))
                            rst = pc3.tile([F, 2], F32, tag="rst")
                            nc.vector.reduce_max(
                                out=rst[:, 0:1], in_=sps[:], axis=AX, negate=True)
                            dist = pc3.tile([F, F], F32R, tag="distt")
                            nc.scalar.activation(
                                out=dist, in_=sps[:], func=EXP,
                                bias=rst[:, 0:1], scale=1.0,
                                accum_out=rst[:, 1:2])
                            nc.vector.reciprocal(out=rst[:, 1:2], in_=rst[:, 1:2])
                            nc.vector.tensor_scalar_mul(
                                out=dist, in0=dist, scalar1=rst[:, 1:2])
                            dTp = ps_attnc.tile([F, F], F32R, tag="attnc")
                            nc.tensor.transpose(
                                dTp[:], dist[:], ident[0:F, 0:F])
                            dT = pc3.tile([F, F], F32R, tag="dTt")
                            nc.scalar.copy(out=dT, in_=dTp[:])
                            # out^T[c, f_q]; gate with beta
                            aps = ps_attnc.tile([128, 2, F], F32, tag="attnc")
                            for oc in (0, 1):
                                nc.tensor.matmul(
                                    aps[:, oc], vT[:, oc, :], dT[:],
                                    start=True, stop=True)
                            bet = pc3.tile([128, 2, F], F32R, tag="bet")
                            nc.sync.dma_start(
                                out=bet,
                                in_=be_d.rearrange("(cc p) b t f -> p cc b t f",
                                                   p=128)[:, :, b, tx - 1, :])
                            for oc in (0, 1):
                                nc.vector.tensor_mul(
                                    out=gt[oc][:, b, tx, 1:1 + F],
                                    in0=aps[:, oc],
                                    in1=bet[:, oc, :])

                # ---- time + x partial final conv ----
                for tg in range(8):
                    r0 = 2 + 4 * tg
                    ops = ps_out.tile([OUT_CH, B, 4, F], F32, tag="fin")
                    n = 0
                    for src, wt in [(gt, wft), (None, wfx)]:
                        for ccin in (0, 1):
                            for i in range(3):
                                for j in range(3):
                                    if src is None:
                                        rhs = xt[:, ccin, :, r0 + i - 1:r0 + i + 3,
                                                 j:j + F]
                                    else:
                                        rhs = src[ccin][:, :, r0 + i - 1:r0 + i + 3,
                                                        j:j + F]
                                    nc.tensor.matmul(
                                        ops[:], wt[:, ccin, 3 * i + j, :], rhs,
                                        start=(n == 0), stop=(n == 35))
                                    n += 1
                    osb = pc3.tile([OUT_CH, B, 4, F], F32R, tag="osbt")
                    nc.vector.tensor_copy(out=osb, in_=ops[:])
                    nc.sync.dma_start(out=ot_d[tg], in_=osb)

    nc.compile()
    return nc


def _prep_inputs(core, x, weights, biases, alpha, beta):
    """Build the per-core input map (all float32, contiguous)."""
    f0 = 8 * core
    t0 = 32 * core

    # xf [C, B, FH, TP]: global f in [f0-1, f0+9), tt = t+1
    xf = np.zeros((C, B, FH, TP), np.float32)
    flo, fhi = max(0, f0 - 1), min(F, f0 + 9)
    xf[:, :, flo - (f0 - 1):fhi - (f0 - 1), 1:1 + T] = \
        x[:, :, flo:fhi, :].transpose(3, 0, 2, 1)

    # xt [C, B, TH, FP]: global t in [t0-2, t0+36), fp = f+1
    xt = np.zeros((C, B, TH, FP), np.float32)
    tlo, thi = max(0, t0 - 2), min(T, t0 + 36)
    xt[:, :, tlo - (t0 - 2):thi - (t0 - 2), 1:1 + F] = \
        x[:, tlo:thi, :, :].transpose(3, 0, 1, 2)

    # al [C, B, NH, TQ]: head fl -> global f0+fl (owned heads, in range)
    al = np.zeros((C, B, NH, TQ), np.float32)
    al[:, :, :, 0:T] = alpha[:, :, f0:f0 + NH, :].transpose(3, 0, 2, 1)

    # be [C, B, NTH, F]: row hl -> global t0-1+hl
    be = np.zeros((C, B, NTH, F), np.float32)
    tl2, th2 = max(0, t0 - 1), min(T, t0 + 33)
    be[:, :, tl2 - (t0 - 1):th2 - (t0 - 1), :] = \
        beta[:, tl2:th2, :, :].transpose(3, 0, 1, 2)

    m = {"xf": xf, "xt": xt, "al": al, "be": be,
         "ident": np.eye(128, dtype=np.float32),
         "zz": np.zeros((128, 256), np.float32)}
    for k, v in weights.items():
        m[k] = v
    for k, v in biases.items():
        m[k] = v
    return {k: np.ascontiguousarray(v) for k, v in m.items()}


def _prep_shared(wq_f, wk_f, wv_f, wq_t, wk_t, wv_t, w_final,
                 bq_f, bk_f, bv_f, bq_t, bk_t, bv_t):
    # channel-major [cin, tap, cout] from [3, 3, cin, cout]
    def cm(w):
        return np.ascontiguousarray(
            w.reshape(9, C, -1).transpose(1, 0, 2).astype(np.float32))
    weights = {"wqf": cm(wq_f), "wkf": cm(wk_f), "wvf": cm(wv_f),
               "wqt": cm(wq_t), "wkt": cm(wk_t), "wvt": cm(wv_t),
               "wff": cm(w_final[:, :, 0:C, :]),
               "wft": cm(w_final[:, :, C:2 * C, :]),
               "wfx": cm(w_final[:, :, 2 * C:3 * C, :])}
    biases = {n: np.ascontiguousarray(b.reshape(2, 128, 1).astype(np.float32))
              for n, b in [("bqf", bq_f), ("bkf", bk_f), ("bvf", bv_f),
                           ("bqt", bq_t), ("bkt", bk_t), ("bvt", bv_t)]}
    return weights, biases


def _assemble(results, b_final):
    out = np.zeros((B, T, F, OUT_CH), np.float32)
    for core, r in enumerate(results):
        of = r["of"]                      # [10, OUT_CH, B, TQ], col f0-1+c0
        ot = r["ot"]                      # [8, OUT_CH, B, 4, F]
        f0, t0 = 8 * core, 32 * core
        clo, chi = max(0, f0 - 1), min(F, f0 + 9)
        out[:, :, clo:chi, :] += of[clo - (f0 - 1):chi - (f0 - 1),
                                    :, :, 0:T].transpose(2, 3, 0, 1)
        thi = min(T, t0 + 32)
        ott = ot.transpose(2, 0, 3, 4, 1).reshape(B, 32, F, OUT_CH)
        out[:, t0:thi, :, :] += ott[:, 0:thi - t0]
    return out + b_final.astype(np.float32)


def kernel(x, wq_f, bq_f, wk_f, bk_f, wv_f, bv_f,
           wq_t, bq_t, wk_t, bk_t, wv_t, bv_t,
           w_final, b_final, alpha, beta):
    from concourse import bass_utils

    if "nc" not in _CACHE:
        _CACHE["nc"] = _build_program()
    nc = _CACHE["nc"]

    weights, biases = _prep_shared(
        np.asarray(wq_f), np.asarray(wk_f), np.asarray(wv_f),
        np.asarray(wq_t), np.asarray(wk_t), np.asarray(wv_t),
        np.asarray(w_final),
        np.asarray(bq_f), np.asarray(bk_f), np.asarray(bv_f),
        np.asarray(bq_t), np.asarray(bk_t), np.asarray(bv_t))
    x = np.asarray(x, np.float32)
    alpha = np.asarray(alpha, np.float32)
    beta = np.asarray(beta, np.float32)

    in_maps = [_prep_inputs(i, x, weights, biases, alpha, beta)
               for i in range(8)]

    if os.environ.get("ATFA_BACKEND") == "sim":
        from concourse.bass_interp import CoreSim
        results = []
        for i in range(8):
            sim = CoreSim(nc, trace=False)
            for k, v in in_maps[i].items():
                sim.tensor(k)[:] = v
            sim.simulate(check_with_hw=False)
            results.append({"of": np.array(sim.tensor("of")),
                            "ot": np.array(sim.tensor("ot"))})
    else:
        res = bass_utils.run_bass_kernel_spmd(
            nc, in_maps, core_ids=list(range(8)),
            trace=bool(int(os.environ.get("ATFA_TRACE", "0"))))
        _CACHE["last_result"] = res
        results = res.results

    return _assemble(results, np.asarray(b_final, np.float32))
